# revision 69
# baseline (speedup 1.0000x reference)
"""Causal multi-head self-attention on 8 Trainium2 NeuronCores (bf16).

Problem: B=4, S=2048, D_MODEL=2048, H=16 heads, d_k=128, RoPE, causal
softmax, fp32 I/O.

Sharding: 8 cores = 4 batches x 2 head-groups (8 heads each).  Each core
computes QKV projections for its head group (weights sharded by output
rows), RoPE, head-local causal attention, and a partial o_proj over its
1024 input features.  The host sums the two partial o_proj outputs per
batch.

v2 design (vs the fp32r baseline; sim 712us -> 590us, ~95% PE busy):
- ALL matmul operands are bf16 (PSUM accumulation stays fp32; output is
  fp32).  Measured end-to-end gate error ~3.7e-3 vs the 2e-2 limit.
  fp8 DoubleRow was evaluated and rejected: quantizing any single GEMM's
  operands to e4m3 already exceeds the error budget (measured 2-4e-2).
- QT/KT/V/attnT are fully SBUF-resident: no DRAM scratch bounce at all,
  which removes the phase-boundary DMA gaps entirely.
- Phase order: Q/K projections + RoPE first (head 0's matmuls pace the
  x DMA stream), then V (x fully resident), then attention fused with
  o_proj.  Within a head, Q's 16 kc matmuls run, then Q's evict+RoPE
  issue while K's matmuls fill the PE, so the PSUM ring never stalls.
- RoPE is 1 ACT eviction + 4 full-height DVE ops per tensor, using a
  duplicated cos table and a sign-flipped [+sin; -sin] table; the
  eviction releases the projection PSUM group immediately.
- Attention k-chunks are processed in PAIRS sharing one 2-bank PSUM
  tile so each exp activation covers 1024 columns (halves ACT op count;
  ACT is the attention-phase near-bottleneck).  ONE software pipeline,
  two pairs deep, runs across all 8 heads of a q-block so per-head
  fill/drain bubbles vanish and ACT stays fed across head seams.
- Softmax denominators never touch the PE: exp tiles accumulate on DVE
  (bf16) and one gpsimd partition_all_reduce per (head, q-block)
  produces the broadcast denominator; DVE reciprocal finishes it.
- Causal diagonal: chunk j=0 gets the triangle mask on the block's
  first 128 cols, j=1 computes full width masked post-exp (keeps exp
  pairable), j=2/3 compute only the 256-col suffix.
- o_proj is interleaved into attention: each q-block's 16 output tiles
  are issued as padding PE work inside the NEXT q-block's attention
  (q-blocks ordered 1,2,3,0), hiding exp latency; evictions alternate
  ACT/DVE.
"""

import sys

for _p in ("/opt/trn_rl_repo", "/root/.axon_site/_ro/trn_rl_repo"):
    if _p not in sys.path:
        sys.path.insert(0, _p)

import numpy as np
import ml_dtypes

import concourse.bacc as bacc
import concourse.bass_isa as bass_isa
import concourse.mybir as mybir
import concourse.tile as tile

F32 = mybir.dt.float32
BF16 = mybir.dt.bfloat16
EXPF = mybir.ActivationFunctionType.Exp
COPYF = mybir.ActivationFunctionType.Copy
MUL = mybir.AluOpType.mult
ADD = mybir.AluOpType.add
SUB = mybir.AluOpType.subtract

D_MODEL = 2048
NUM_HEADS = 16
D_K = 128
ROPE_THETA = 10000.0
B = 4
S = 2048
N_CORES = 8
GROUPS = 2  # head groups (tensor parallel factor)
H_LOC = NUM_HEADS // GROUPS  # heads per core

NP_BF16 = ml_dtypes.bfloat16


def build_nc(D=D_MODEL, S_=S, H_loc=H_LOC, QB=512):
    P = 128
    DK = 128
    E = H_loc * DK
    KCN = D // P
    NSB = S_ // P
    NQB = S_ // QB
    NDIAG = QB // P  # 4 k-chunks per q block on the diagonal
    SCALE = 1.0 / float(np.sqrt(DK))

    nc = bacc.Bacc("TRN2", target_bir_lowering=False, debug=False,
                   num_devices=N_CORES)

    xT = nc.dram_tensor("xT", [D, S_], BF16, kind="ExternalInput")
    # wq/wk are laid out [head, partition, kc*dk] host-side so each head's
    # slice is one 4KB-per-partition contiguous DMA (256B-row slices of a
    # [D, E] layout run at less than half DMA efficiency)
    wqH = nc.dram_tensor("wqH", [H_loc, P, (D // P) * DK], BF16,
                         kind="ExternalInput")
    wkH = nc.dram_tensor("wkH", [H_loc, P, (D // P) * DK], BF16,
                         kind="ExternalInput")
    wvT = nc.dram_tensor("wvT", [D, E], BF16, kind="ExternalInput")
    woT = nc.dram_tensor("woT", [E, D], BF16, kind="ExternalInput")
    # RoPE tables, de-interleaved head layout (even dims rows 0..63, odd
    # dims rows 64..127; wq/wk cols permuted host-side).  cosH duplicates
    # the half-table to full d_k height; sinN is [+sin; -sin] so the
    # rotation reduces to raw*cos + swap(raw)*sinN (see rope_evict).
    cosH = nc.dram_tensor("cosH", [DK, S_], BF16, kind="ExternalInput")
    sinN = nc.dram_tensor("sinN", [DK, S_], BF16, kind="ExternalInput")
    # [P, 256]: cols 0..127 all-zero, cols 128..255 causal triangle
    maskE = nc.dram_tensor("maskE", [P, 2 * P], BF16, kind="ExternalInput")
    out = nc.dram_tensor("out", [S_, D], F32, kind="ExternalOutput")

    xT_t = xT.rearrange("(kc p) s -> p kc s", p=P)
    wq_t = wqH.rearrange("h p (kc dk) -> h p kc dk", dk=DK)
    wk_t = wkH.rearrange("h p (kc dk) -> h p kc dk", dk=DK)
    wv_t = wvT.rearrange("(kc p) e -> p kc e", p=P)
    woT_t = woT.rearrange("(ec p) n -> p ec n", p=P)

    HH = DK // 2  # 64

    with tile.TileContext(nc) as tc:
        with (
            tc.tile_pool(name="const", bufs=1) as const,
            tc.tile_pool(name="res", bufs=1) as res,
        ):
            maskE_sb = const.tile([P, 2 * P], BF16)
            warm = const.tile([1, 8], F32)

            # SBUF-resident rotated QT/KT ([dk, h, S]) and V ([s, sb, e])
            qt_res = res.tile([P, H_loc, S_], BF16)
            kt_res = res.tile([P, H_loc, S_], BF16)
            v_res = res.tile([P, NSB, E], BF16)
            # early-loaded head of wv (first 2 kc chunks) so phase 1b's
            # first matmuls don't wait for the post-1a wv stream
            KCH = min(4, KCN)
            wv_head = res.tile([P, KCH, E], BF16)

            with tc.tile_pool(name="xres", bufs=1) as xres:
                x_res = xres.tile([P, KCN, S_], BF16)

                # ---- Phase 1a: Q/K head-transposed projections + RoPE ----
                # Runs FIRST so its matmuls pace the x DMA stream.  Head 0
                # interleaves Q and K per kc so PE work (~1.7us/kc) covers
                # the x chunk arrival (~1.4us/kc).
                with (
                    tc.tile_pool(name="trig", bufs=1) as trig,
                    tc.tile_pool(name="wslice", bufs=2) as wslice,
                    tc.tile_pool(name="qk_ps", bufs=2, space="PSUM") as qk_ps,
                    tc.tile_pool(name="rawp", bufs=2) as rawp,
                    tc.tile_pool(name="ropet", bufs=1) as ropet,
                ):
                    cos_sb = trig.tile([DK, S_], BF16)
                    sin_sb = trig.tile([DK, S_], BF16)

                    def load_w(h):
                        wq_sl = wslice.tile([P, KCN, DK], BF16, tag="wq",
                                            name=f"wq_{h}")
                        wk_sl = wslice.tile([P, KCN, DK], BF16, tag="wk",
                                            name=f"wk_{h}")
                        nc.sync.dma_start(wq_sl[:], wq_t[h])
                        nc.sync.dma_start(wk_sl[:], wk_t[h])
                        return wq_sl, wk_sl

                    # issue order feeds the head-0 Q matmul soonest: its
                    # stationary weight, then x chunk 0, then the K weight
                    wq_sl0 = wslice.tile([P, KCN, DK], BF16, tag="wq",
                                         name="wq_0")
                    nc.sync.dma_start(wq_sl0[:], wq_t[0])
                    nc.sync.dma_start(x_res[:, 0], xT_t[:, 0])
                    wk_sl0 = wslice.tile([P, KCN, DK], BF16, tag="wk",
                                         name="wk_0")
                    nc.sync.dma_start(wk_sl0[:], wk_t[0])
                    w_cur = (wq_sl0, wk_sl0)
                    ntrig = min(4, KCN)
                    for kc in range(1, ntrig):
                        nc.sync.dma_start(x_res[:, kc], xT_t[:, kc])
                    nc.sync.dma_start(cos_sb[:], cosH[:])
                    nc.sync.dma_start(sin_sb[:], sinN[:])
                    for kc in range(ntrig, KCN):
                        nc.sync.dma_start(x_res[:, kc], xT_t[:, kc])
                    for kc in range(KCH):
                        nc.sync.dma_start(wv_head[:, kc], wv_t[:, kc])
                    nc.sync.dma_start(maskE_sb[:], maskE[:])
                    # touch Exp once so the ACT function-table load (~1.3us)
                    # happens here instead of at the first attention softmax
                    nc.scalar.activation(warm[:], maskE_sb[0:1, 0:8], EXPF)

                    def rope_evict(pgrp, dst, h):
                        # Single ACT eviction releases the PSUM group right
                        # away; rotation is 4 full-height DVE ops using the
                        # duplicated cos table and the [+sin; -sin] table:
                        #   t2 = swap64(raw) * sinN -> [-O*s; E*s]
                        #   raw *= cos (in place)   -> [E*c; O*c]
                        #   dst = raw + t2          -> [rotE; rotO]
                        raw = rawp.tile([DK, S_], BF16, tag="raw")
                        raw_v = raw[:].rearrange("p (a b) -> p a b", b=1024)
                        pg_v = pgrp[:].rearrange("p (a b) -> p a b", b=1024)
                        nc.scalar.activation(raw_v, pg_v, COPYF)
                        t2 = ropet.tile([DK, S_], BF16, tag="tmp")
                        nc.vector.tensor_tensor(
                            t2[:HH], raw[HH:], sin_sb[HH:], MUL)
                        nc.vector.tensor_tensor(
                            t2[HH:], raw[:HH], sin_sb[:HH], MUL)
                        nc.vector.tensor_tensor(
                            raw[:], raw[:], cos_sb[:], MUL)
                        nc.vector.tensor_tensor(
                            dst[:, h], raw[:], t2[:], ADD)

                    def proj_mms(pg, w_sl):
                        for kc in range(KCN):
                            for st in range(S_ // 512):
                                nc.tensor.matmul(
                                    pg[:, st * 512:(st + 1) * 512],
                                    w_sl[:, kc],
                                    x_res[:, kc, st * 512:(st + 1) * 512],
                                    start=(kc == 0), stop=(kc == KCN - 1))

                    for h in range(H_loc):
                        wq_sl, wk_sl = w_cur
                        if h + 1 < H_loc:
                            w_next = load_w(h + 1)
                        pq = qk_ps.tile([P, S_], F32, tag="qk", name=f"pq_{h}")
                        pk = qk_ps.tile([P, S_], F32, tag="qk", name=f"pk_{h}")
                        if h == 0:
                            # head 0 interleaves Q/K per kc so PE work paces
                            # the x DMA stream without idling; the first two
                            # kc use the tiny fast-start weight copies
                            for kc in range(KCN):
                                for pg, w_sl in ((pq, wq_sl), (pk, wk_sl)):
                                    for st in range(S_ // 512):
                                        nc.tensor.matmul(
                                            pg[:, st * 512:(st + 1) * 512],
                                            w_sl[:, kc],
                                            x_res[:, kc,
                                                  st * 512:(st + 1) * 512],
                                            start=(kc == 0),
                                            stop=(kc == KCN - 1))
                            rope_evict(pq, qt_res, h)
                            rope_evict(pk, kt_res, h)
                        else:
                            # later heads run Q fully, evict+rope it while
                            # K's matmuls fill the PE, so the PSUM ring and
                            # raw/rope chain never block the next head
                            proj_mms(pq, wq_sl)
                            rope_evict(pq, qt_res, h)
                            proj_mms(pk, wk_sl)
                            rope_evict(pk, kt_res, h)
                        if h + 1 < H_loc:
                            w_cur = w_next

                # ---- Phase 1b: V via x-stationary matmuls (x resident) ----
                with (
                    tc.tile_pool(name="wv", bufs=1) as wvp,
                    tc.tile_pool(name="v_ps", bufs=4, space="PSUM") as v_ps,
                ):
                    wv_tail = None
                    if KCN > KCH:
                        wv_tail = wvp.tile([P, KCN - KCH, E], BF16)
                        for kc in range(KCN - KCH):
                            nc.sync.dma_start(wv_tail[:, kc],
                                              wv_t[:, KCH + kc])
                    EH = min(512, E)
                    for g in range(NSB // 4):
                        psv = [v_ps.tile([P, E], F32, tag="vps",
                                         name=f"vps_{g}_{i}") for i in range(4)]
                        for kc in range(KCN):
                            wv_kc = (wv_head[:, kc] if kc < KCH
                                     else wv_tail[:, kc - KCH])
                            for i in range(4):
                                sb = g * 4 + i
                                for eh in range(E // EH):
                                    nc.tensor.matmul(
                                        psv[i][:, eh * EH:(eh + 1) * EH],
                                        x_res[:, kc, sb * P:(sb + 1) * P],
                                        wv_kc[:, eh * EH:(eh + 1) * EH],
                                        start=(kc == 0), stop=(kc == KCN - 1))
                        for i in range(4):
                            # alternate eviction engines so the group
                            # boundary (and the V->attention hand-off)
                            # drains two evictions at once
                            if i % 2:
                                nc.vector.tensor_scalar_mul(
                                    v_res[:, g * 4 + i], psv[i][:], 1.0)
                            else:
                                nc.scalar.activation(v_res[:, g * 4 + i],
                                                     psv[i][:], COPYF)

            # ---------------- Phase 2: attention -----------------
            with (
                tc.tile_pool(name="attnT", bufs=1) as attnT_pool,
                tc.tile_pool(name="wo", bufs=1) as wo_pool,
            ):
                attnT = attnT_pool.tile([DK, H_loc, S_], BF16)
                wo_sb = wo_pool.tile([P, H_loc, D], BF16)
                nc.sync.dma_start(wo_sb[:], woT_t[:])

                with (
                    tc.tile_pool(name="expt", bufs=6) as expt,
                    tc.tile_pool(name="accp", bufs=2) as accp,
                    tc.tile_pool(name="sc_ps", bufs=2, space="PSUM") as sc_ps,
                    tc.tile_pool(name="pv_ps", bufs=2, space="PSUM") as pv_ps,
                    tc.tile_pool(name="op_ps", bufs=2, space="PSUM") as op_ps,
                    tc.tile_pool(name="inv", bufs=2) as invp,
                    tc.tile_pool(name="osb", bufs=3) as osb,
                ):
                    M_ext = maskE_sb[:, 0:2 * P]   # zeros | triangle
                    M_tri = maskE_sb[:, P:2 * P]   # triangle only

                    # o_proj for one (s-block, 512-col) output tile; these
                    # units interleave into the NEXT q-block's attention so
                    # the PE never starves while ACT works through the exps
                    NT = min(512, D)
                    def oproj_unit(sb_i, nt):
                        ps = op_ps.tile([P, NT], F32, tag="op",
                                        name=f"op_{sb_i}_{nt}")
                        for ec in range(H_loc):
                            nc.tensor.matmul(
                                ps[:],
                                attnT[:, ec, sb_i * P:(sb_i + 1) * P],
                                wo_sb[:, ec, nt * NT:(nt + 1) * NT],
                                start=(ec == 0), stop=(ec == H_loc - 1))
                        o_nt = osb.tile([P, NT], F32, tag="osb",
                                        name=f"osb_{sb_i}_{nt}")
                        # alternate eviction engines: keeps ACT (exp-bound
                        # on HW) relieved without serializing behind DVE
                        if (sb_i + nt) % 2:
                            nc.vector.tensor_scalar_mul(o_nt[:], ps[:], 1.0)
                        else:
                            nc.scalar.activation(o_nt[:], ps[:], COPYF)
                        nc.sync.dma_start(
                            out[sb_i * P:(sb_i + 1) * P, nt * NT:(nt + 1) * NT],
                            o_nt[:])

                    pending = []
                    drain_k = max(1, -(-((QB // P) * (D // NT)) // H_loc))

                    # qb 0 last: its heads have the least attention work per
                    # head, so they benefit most from o_proj padding
                    qb_order = list(range(1, NQB)) + [0]
                    for qb in qb_order:
                        kcn = (qb + 1) * NDIAG
                        npair = kcn // 2
                        q_lo = qb * QB
                        # per-head accumulation state, created lazily
                        hstate = {}

                        def ensure(h):
                            if h not in hstate:
                                hstate[h] = (
                                    pv_ps.tile([P, QB], F32, tag="pv",
                                               name=f"pv_{qb}_{h}"),
                                    accp.tile([P, QB], BF16, tag="acc",
                                              name=f"acc_{qb}_{h}"))
                            return hstate[h]

                        # pair p covers chunks (2p, 2p+1).
                        #  p <  2*qb : both full
                        #  p == 2*qb : A = (j0 full, j1 full-width+mask)
                        #  p == 2*qb+1: B = (j2, j3) suffix [256:512)
                        def scores_exp(h, p):
                            is_b = (p == 2 * qb + 1)
                            ps_s = sc_ps.tile([P, 2, QB], F32, tag="sc",
                                              name=f"ss_{h}_{qb}_{p}")
                            e_t = expt.tile([P, 2, QB], BF16, tag="e",
                                            name=f"e_{h}_{qb}_{p}")
                            off = 2 * P if is_b else 0
                            for i in range(2):
                                kc = 2 * p + i
                                nc.tensor.matmul(
                                    ps_s[:, i, off:],
                                    kt_res[:, h, kc * P:(kc + 1) * P],
                                    qt_res[:, h, q_lo + off:q_lo + QB],
                                    start=True, stop=True)
                            nc.scalar.activation(e_t[:, :, off:],
                                                 ps_s[:, :, off:], EXPF,
                                                 scale=SCALE)
                            if p == 2 * qb:
                                # pair A: j=0 half gets the triangle on the
                                # block's first 128 cols; j=1 half gets
                                # zeros+triangle on the first 256
                                nc.vector.tensor_tensor(
                                    e_t[:, 0, 0:P], e_t[:, 0, 0:P],
                                    M_tri, MUL)
                                nc.vector.tensor_tensor(
                                    e_t[:, 1, 0:2 * P], e_t[:, 1, 0:2 * P],
                                    M_ext, MUL)
                            elif is_b:
                                nc.vector.tensor_tensor(
                                    e_t[:, 0, 2 * P:3 * P],
                                    e_t[:, 0, 2 * P:3 * P], M_tri, MUL)
                                nc.vector.tensor_tensor(
                                    e_t[:, 1, 2 * P:], e_t[:, 1, 2 * P:],
                                    M_ext, MUL)
                            return e_t

                        def consume(h, p, e_t):
                            # denominator accumulation (DVE, bf16) + PV
                            ps_pv, acc = ensure(h)
                            is_b = (p == 2 * qb + 1)
                            if p == 0:
                                nc.vector.tensor_tensor(
                                    acc[:], e_t[:, 0], e_t[:, 1], ADD)
                            elif is_b:
                                for i in range(2):
                                    nc.vector.tensor_tensor(
                                        acc[:, 2 * P:], acc[:, 2 * P:],
                                        e_t[:, i, 2 * P:], ADD)
                            else:
                                for i in range(2):
                                    nc.vector.tensor_tensor(
                                        acc[:], acc[:], e_t[:, i], ADD)
                            for i in range(2):
                                kc = 2 * p + i
                                v_sl = v_res[:, kc, h * DK:(h + 1) * DK]
                                if is_b:
                                    nc.tensor.matmul(
                                        ps_pv[:, 2 * P:], v_sl,
                                        e_t[:, i, 2 * P:],
                                        start=False, stop=(kc == kcn - 1))
                                else:
                                    nc.tensor.matmul(
                                        ps_pv[:], v_sl, e_t[:, i],
                                        start=(kc == 0), stop=(kc == kcn - 1))
                            if p == npair - 1:
                                finalize(h)

                        def finalize(h):
                            ps_pv, acc = hstate.pop(h)
                            den_b = invp.tile([P, QB], F32, tag="den",
                                              name=f"den_{h}_{qb}")
                            nc.gpsimd.partition_all_reduce(
                                den_b[:], acc[:], channels=P,
                                reduce_op=bass_isa.ReduceOp.add)
                            inv_b = invp.tile([P, QB], F32, tag="invb",
                                              name=f"inv_{h}_{qb}")
                            nc.vector.reciprocal(inv_b[:], den_b[:])
                            nc.vector.tensor_tensor(
                                attnT[:, h, q_lo:q_lo + QB], ps_pv[:],
                                inv_b[:], MUL)

                        # ONE software pipeline across all heads of this
                        # q-block (two pairs deep), so per-head fill/drain
                        # bubbles vanish; o_proj units of the previous
                        # q-block drain at head starts as ready PE work
                        pipe = []
                        for h in range(H_loc):
                            for p in range(npair):
                                pipe.append((h, p, scores_exp(h, p)))
                                if p == min(1, npair - 1):
                                    # drain behind the freshly issued scores
                                    # so ACT has exp work while PE does these
                                    for _ in range(drain_k):
                                        if pending:
                                            oproj_unit(*pending.pop(0))
                                if len(pipe) > 2:
                                    hh, pp, e_t = pipe.pop(0)
                                    consume(hh, pp, e_t)
                        for hh, pp, e_t in pipe:
                            consume(hh, pp, e_t)

                        # queue this q-block's o_proj output tiles
                        pending.extend(
                            (sb_i, nt)
                            for sb_i in range(qb * (QB // P),
                                              (qb + 1) * (QB // P))
                            for nt in range(D // NT))
                    for u in pending:
                        oproj_unit(*u)

    nc.compile()
    return nc


def make_tables(token_positions, S_=S, DK=D_K):
    """Host-side RoPE tables (de-interleaved halves) + extended causal mask.

    cosH = [cos; cos]; sinN = [+sin; -sin] (see rope_evict).
    """
    pos = np.asarray(token_positions).astype(np.float64)
    half = np.arange(0, DK, 2, dtype=np.float64) / DK
    inv_freq = 1.0 / (ROPE_THETA ** half)  # [DK/2]
    ang = pos[:, None] * inv_freq[None, :]  # [S, DK/2]
    c = np.cos(ang).T.astype(NP_BF16)  # [DK/2, S]
    s = np.sin(ang).T.astype(NP_BF16)
    cosH = np.ascontiguousarray(np.concatenate([c, c], axis=0))  # [DK, S]
    sinN = np.ascontiguousarray(np.concatenate([s, -s], axis=0))
    kl = np.arange(128)[:, None]
    ql = np.arange(128)[None, :]
    tri = (ql >= kl).astype(NP_BF16)  # [128, 128] causal triangle
    maskE = np.concatenate([np.zeros((128, 128), NP_BF16), tri], axis=1)
    return cosH, sinN, np.ascontiguousarray(maskE)


# de-interleave permutation within each head's 128 dims: even dims first
_DEINT = np.concatenate([np.arange(0, D_K, 2), np.arange(1, D_K, 2)])


def deinterleave_cols(wT, n_heads):
    """Permute per-head output columns of a [D, n_heads*DK] matrix so even
    RoPE dims land in rows 0..63 of the head-transposed projection."""
    w = np.asarray(wT)
    out = np.empty_like(w)
    for h in range(n_heads):
        out[:, h * D_K:(h + 1) * D_K] = w[:, h * D_K + _DEINT]
    return out


def head_major(wT, n_heads, d=D_MODEL):
    """[D, E] -> [H, P, KCN*DK]: per-head, partition-major contiguous."""
    kcn = d // 128
    return np.ascontiguousarray(
        wT.reshape(kcn, 128, n_heads, D_K).transpose(2, 1, 0, 3)
        .reshape(n_heads, 128, kcn * D_K))


def make_in_maps(x, token_positions, q_w, k_w, v_w, o_w):
    cosH, sinN, maskE = make_tables(token_positions)
    x = np.asarray(x, np.float32)
    in_maps = []
    for c in range(N_CORES):
        b, g = c // GROUPS, c % GROUPS
        e_lo, e_hi = g * H_LOC * D_K, (g + 1) * H_LOC * D_K
        wqT = deinterleave_cols(np.asarray(q_w, np.float32)[e_lo:e_hi, :].T, H_LOC)
        wkT = deinterleave_cols(np.asarray(k_w, np.float32)[e_lo:e_hi, :].T, H_LOC)
        wqHm = head_major(wqT, H_LOC).astype(NP_BF16)
        wkHm = head_major(wkT, H_LOC).astype(NP_BF16)
        in_maps.append({
            "xT": np.ascontiguousarray(x[b].T).astype(NP_BF16),
            "wqH": wqHm,
            "wkH": wkHm,
            "wvT": np.ascontiguousarray(np.asarray(v_w, np.float32)[e_lo:e_hi, :].T).astype(NP_BF16),
            "woT": np.ascontiguousarray(np.asarray(o_w, np.float32)[:, e_lo:e_hi].T).astype(NP_BF16),
            "cosH": cosH,
            "sinN": sinN,
            "maskE": maskE,
        })
    return in_maps


_NC_CACHE = None


def get_nc():
    global _NC_CACHE
    if _NC_CACHE is None:
        _NC_CACHE = build_nc(D_MODEL, S, H_LOC)
    return _NC_CACHE


def kernel(x, token_positions, q_w, k_w, v_w, o_w):
    from concourse.bass_utils import run_bass_kernel_spmd

    nc = get_nc()
    in_maps = make_in_maps(x, token_positions, q_w, k_w, v_w, o_w)
    res = run_bass_kernel_spmd(nc, in_maps, list(range(N_CORES)))
    outs = [res.results[c]["out"] for c in range(N_CORES)]
    full = np.empty((B, S, D_MODEL), np.float32)
    for b in range(B):
        full[b] = outs[GROUPS * b]
        for g in range(1, GROUPS):
            full[b] += outs[GROUPS * b + g]
    return full


# revision 70
# speedup vs baseline: 1.5626x; 1.5626x over previous
"""Causal multi-head self-attention on 8 Trainium2 NeuronCores (bf16).

Problem: B=4, S=2048, D_MODEL=2048, H=16 heads, d_k=128, RoPE, causal
softmax, fp32 I/O.

Sharding: 8 cores = 4 batches x 2 head-groups (8 heads each).  Each core
computes QKV projections for its head group (weights sharded by output
rows), RoPE, head-local causal attention, and a partial o_proj over its
1024 input features.  The host sums the two partial o_proj outputs per
batch.

v2 design (vs the fp32r baseline; sim 712us -> 590us, ~95% PE busy):
- ALL matmul operands are bf16 (PSUM accumulation stays fp32; output is
  fp32).  Measured end-to-end gate error ~3.7e-3 vs the 2e-2 limit.
  fp8 DoubleRow was evaluated and rejected: quantizing any single GEMM's
  operands to e4m3 already exceeds the error budget (measured 2-4e-2).
- QT/KT/V/attnT are fully SBUF-resident: no DRAM scratch bounce at all,
  which removes the phase-boundary DMA gaps entirely.
- Phase order: Q/K projections + RoPE first (head 0's matmuls pace the
  x DMA stream), then V (x fully resident), then attention fused with
  o_proj.  Within a head, Q's 16 kc matmuls run, then Q's evict+RoPE
  issue while K's matmuls fill the PE, so the PSUM ring never stalls.
- RoPE is 1 ACT eviction + 4 full-height DVE ops per tensor, using a
  duplicated cos table and a sign-flipped [+sin; -sin] table; the
  eviction releases the projection PSUM group immediately.
- Attention k-chunks are processed in PAIRS sharing one 2-bank PSUM
  tile so each exp activation covers 1024 columns (halves ACT op count;
  ACT is the attention-phase near-bottleneck).  ONE software pipeline,
  two pairs deep, runs across all 8 heads of a q-block so per-head
  fill/drain bubbles vanish and ACT stays fed across head seams.
- Softmax denominators never touch the PE: exp tiles accumulate on DVE
  (bf16) and one gpsimd partition_all_reduce per (head, q-block)
  produces the broadcast denominator; DVE reciprocal finishes it.
- Causal diagonal: chunk j=0 gets the triangle mask on the block's
  first 128 cols, j=1 computes full width masked post-exp (keeps exp
  pairable), j=2/3 compute only the 256-col suffix.
- o_proj is interleaved into attention: each q-block's 16 output tiles
  are issued as padding PE work inside the NEXT q-block's attention
  (q-blocks ordered 1,2,3,0), hiding exp latency; evictions alternate
  ACT/DVE.
"""

import sys

for _p in ("/opt/trn_rl_repo", "/root/.axon_site/_ro/trn_rl_repo"):
    if _p not in sys.path:
        sys.path.insert(0, _p)

import numpy as np
import ml_dtypes

import concourse.bacc as bacc
import concourse.bass_isa as bass_isa
import concourse.mybir as mybir
import concourse.tile as tile

F32 = mybir.dt.float32
BF16 = mybir.dt.bfloat16
EXPF = mybir.ActivationFunctionType.Exp
COPYF = mybir.ActivationFunctionType.Copy
MUL = mybir.AluOpType.mult
ADD = mybir.AluOpType.add
SUB = mybir.AluOpType.subtract

D_MODEL = 2048
NUM_HEADS = 16
D_K = 128
ROPE_THETA = 10000.0
B = 4
S = 2048
N_CORES = 8
GROUPS = 2  # head groups (tensor parallel factor)
H_LOC = NUM_HEADS // GROUPS  # heads per core

NP_BF16 = ml_dtypes.bfloat16


def build_nc(D=D_MODEL, S_=S, H_loc=H_LOC, QB=512):
    P = 128
    DK = 128
    E = H_loc * DK
    KCN = D // P
    NSB = S_ // P
    NQB = S_ // QB
    NDIAG = QB // P  # 4 k-chunks per q block on the diagonal
    SCALE = 1.0 / float(np.sqrt(DK))

    nc = bacc.Bacc("TRN2", target_bir_lowering=False, debug=False,
                   num_devices=N_CORES)

    xT = nc.dram_tensor("xT", [D, S_], BF16, kind="ExternalInput")
    # wq/wk are laid out [head, partition, kc*dk] host-side so each head's
    # slice is one 4KB-per-partition contiguous DMA (256B-row slices of a
    # [D, E] layout run at less than half DMA efficiency)
    wqH = nc.dram_tensor("wqH", [H_loc, P, (D // P) * DK], BF16,
                         kind="ExternalInput")
    wkH = nc.dram_tensor("wkH", [H_loc, P, (D // P) * DK], BF16,
                         kind="ExternalInput")
    wvT = nc.dram_tensor("wvT", [D, E], BF16, kind="ExternalInput")
    woT = nc.dram_tensor("woT", [E, D], BF16, kind="ExternalInput")
    # RoPE tables, de-interleaved head layout (even dims rows 0..63, odd
    # dims rows 64..127; wq/wk cols permuted host-side).  cosH duplicates
    # the half-table to full d_k height; sinN is [+sin; -sin] so the
    # rotation reduces to raw*cos + swap(raw)*sinN (see rope_evict).
    cosH = nc.dram_tensor("cosH", [DK, S_], BF16, kind="ExternalInput")
    sinN = nc.dram_tensor("sinN", [DK, S_], BF16, kind="ExternalInput")
    # [P, 256]: cols 0..127 all-zero, cols 128..255 causal triangle
    maskE = nc.dram_tensor("maskE", [P, 2 * P], BF16, kind="ExternalInput")
    out = nc.dram_tensor("out", [S_, D], F32, kind="ExternalOutput")

    xT_t = xT.rearrange("(kc p) s -> p kc s", p=P)
    wq_t = wqH.rearrange("h p (kc dk) -> h p kc dk", dk=DK)
    wk_t = wkH.rearrange("h p (kc dk) -> h p kc dk", dk=DK)
    wv_t = wvT.rearrange("(kc p) e -> p kc e", p=P)
    woT_t = woT.rearrange("(ec p) n -> p ec n", p=P)

    HH = DK // 2  # 64

    with tile.TileContext(nc) as tc:
        with (
            tc.tile_pool(name="const", bufs=1) as const,
            tc.tile_pool(name="res", bufs=1) as res,
        ):
            maskE_sb = const.tile([P, 2 * P], BF16)
            warm = const.tile([1, 8], F32)

            # SBUF-resident rotated QT/KT ([dk, h, S]) and V ([s, sb, e])
            qt_res = res.tile([P, H_loc, S_], BF16)
            kt_res = res.tile([P, H_loc, S_], BF16)
            v_res = res.tile([P, NSB, E], BF16)
            # early-loaded head of wv (first 2 kc chunks) so phase 1b's
            # first matmuls don't wait for the post-1a wv stream
            KCH = min(4, KCN)
            wv_head = res.tile([P, KCH, E], BF16)

            with tc.tile_pool(name="xres", bufs=1) as xres:
                x_res = xres.tile([P, KCN, S_], BF16)

                # ---- Phase 1a: Q/K head-transposed projections + RoPE ----
                # Runs FIRST so its matmuls pace the x DMA stream.  Head 0
                # interleaves Q and K per kc so PE work (~1.7us/kc) covers
                # the x chunk arrival (~1.4us/kc).
                with (
                    tc.tile_pool(name="trig", bufs=1) as trig,
                    tc.tile_pool(name="wslice", bufs=2) as wslice,
                    tc.tile_pool(name="qk_ps", bufs=2, space="PSUM") as qk_ps,
                    tc.tile_pool(name="rawp", bufs=2) as rawp,
                    tc.tile_pool(name="ropet", bufs=1) as ropet,
                ):
                    cos_sb = trig.tile([DK, S_], BF16)
                    sin_sb = trig.tile([DK, S_], BF16)

                    def load_w(h):
                        wq_sl = wslice.tile([P, KCN, DK], BF16, tag="wq",
                                            name=f"wq_{h}")
                        wk_sl = wslice.tile([P, KCN, DK], BF16, tag="wk",
                                            name=f"wk_{h}")
                        nc.sync.dma_start(wq_sl[:], wq_t[h])
                        nc.sync.dma_start(wk_sl[:], wk_t[h])
                        return wq_sl, wk_sl

                    # issue order feeds the head-0 Q matmul soonest: its
                    # stationary weight, then x chunk 0, then the K weight
                    wq_sl0 = wslice.tile([P, KCN, DK], BF16, tag="wq",
                                         name="wq_0")
                    nc.sync.dma_start(wq_sl0[:], wq_t[0])
                    nc.sync.dma_start(x_res[:, 0], xT_t[:, 0])
                    wk_sl0 = wslice.tile([P, KCN, DK], BF16, tag="wk",
                                         name="wk_0")
                    nc.sync.dma_start(wk_sl0[:], wk_t[0])
                    w_cur = (wq_sl0, wk_sl0)
                    ntrig = min(4, KCN)
                    for kc in range(1, ntrig):
                        nc.sync.dma_start(x_res[:, kc], xT_t[:, kc])
                    nc.sync.dma_start(cos_sb[:], cosH[:])
                    nc.sync.dma_start(sin_sb[:], sinN[:])
                    for kc in range(ntrig, KCN):
                        nc.sync.dma_start(x_res[:, kc], xT_t[:, kc])
                    for kc in range(KCH):
                        nc.sync.dma_start(wv_head[:, kc], wv_t[:, kc])
                    nc.sync.dma_start(maskE_sb[:], maskE[:])
                    # touch Exp once so the ACT function-table load (~1.3us)
                    # happens here instead of at the first attention softmax
                    nc.scalar.activation(warm[:], maskE_sb[0:1, 0:8], EXPF)

                    def rope_evict(pgrp, dst, h):
                        # Single ACT eviction releases the PSUM group right
                        # away; rotation is 4 full-height DVE ops using the
                        # duplicated cos table and the [+sin; -sin] table:
                        #   t2 = swap64(raw) * sinN -> [-O*s; E*s]
                        #   raw *= cos (in place)   -> [E*c; O*c]
                        #   dst = raw + t2          -> [rotE; rotO]
                        raw = rawp.tile([DK, S_], BF16, tag="raw")
                        raw_v = raw[:].rearrange("p (a b) -> p a b", b=1024)
                        pg_v = pgrp[:].rearrange("p (a b) -> p a b", b=1024)
                        nc.scalar.activation(raw_v, pg_v, COPYF)
                        t2 = ropet.tile([DK, S_], BF16, tag="tmp")
                        nc.vector.tensor_tensor(
                            t2[:HH], raw[HH:], sin_sb[HH:], MUL)
                        nc.vector.tensor_tensor(
                            t2[HH:], raw[:HH], sin_sb[:HH], MUL)
                        nc.vector.tensor_tensor(
                            raw[:], raw[:], cos_sb[:], MUL)
                        nc.vector.tensor_tensor(
                            dst[:, h], raw[:], t2[:], ADD)

                    def proj_mms(pg, w_sl):
                        for kc in range(KCN):
                            for st in range(S_ // 512):
                                nc.tensor.matmul(
                                    pg[:, st * 512:(st + 1) * 512],
                                    w_sl[:, kc],
                                    x_res[:, kc, st * 512:(st + 1) * 512],
                                    start=(kc == 0), stop=(kc == KCN - 1))

                    for h in range(H_loc):
                        wq_sl, wk_sl = w_cur
                        if h + 1 < H_loc:
                            w_next = load_w(h + 1)
                        pq = qk_ps.tile([P, S_], F32, tag="qk", name=f"pq_{h}")
                        pk = qk_ps.tile([P, S_], F32, tag="qk", name=f"pk_{h}")
                        if h == 0:
                            # head 0 interleaves Q/K per kc so PE work paces
                            # the x DMA stream without idling; the first two
                            # kc use the tiny fast-start weight copies
                            for kc in range(KCN):
                                for pg, w_sl in ((pq, wq_sl), (pk, wk_sl)):
                                    for st in range(S_ // 512):
                                        nc.tensor.matmul(
                                            pg[:, st * 512:(st + 1) * 512],
                                            w_sl[:, kc],
                                            x_res[:, kc,
                                                  st * 512:(st + 1) * 512],
                                            start=(kc == 0),
                                            stop=(kc == KCN - 1))
                            rope_evict(pq, qt_res, h)
                            rope_evict(pk, kt_res, h)
                        else:
                            # later heads run Q fully, evict+rope it while
                            # K's matmuls fill the PE, so the PSUM ring and
                            # raw/rope chain never block the next head
                            proj_mms(pq, wq_sl)
                            rope_evict(pq, qt_res, h)
                            proj_mms(pk, wk_sl)
                            rope_evict(pk, kt_res, h)
                        if h + 1 < H_loc:
                            w_cur = w_next

                # ---- Phase 1b: V via x-stationary matmuls (x resident) ----
                with (
                    tc.tile_pool(name="wv", bufs=1) as wvp,
                    tc.tile_pool(name="v_ps", bufs=4, space="PSUM") as v_ps,
                ):
                    wv_tail = None
                    if KCN > KCH:
                        wv_tail = wvp.tile([P, KCN - KCH, E], BF16)
                        for kc in range(KCN - KCH):
                            nc.sync.dma_start(wv_tail[:, kc],
                                              wv_t[:, KCH + kc])
                    EH = min(512, E)
                    for g in range(NSB // 4):
                        psv = [v_ps.tile([P, E], F32, tag="vps",
                                         name=f"vps_{g}_{i}") for i in range(4)]
                        for kc in range(KCN):
                            wv_kc = (wv_head[:, kc] if kc < KCH
                                     else wv_tail[:, kc - KCH])
                            for i in range(4):
                                sb = g * 4 + i
                                for eh in range(E // EH):
                                    nc.tensor.matmul(
                                        psv[i][:, eh * EH:(eh + 1) * EH],
                                        x_res[:, kc, sb * P:(sb + 1) * P],
                                        wv_kc[:, eh * EH:(eh + 1) * EH],
                                        start=(kc == 0), stop=(kc == KCN - 1))
                        for i in range(4):
                            # alternate eviction engines so the group
                            # boundary (and the V->attention hand-off)
                            # drains two evictions at once
                            if i % 2:
                                nc.vector.tensor_scalar_mul(
                                    v_res[:, g * 4 + i], psv[i][:], 1.0)
                            else:
                                nc.scalar.activation(v_res[:, g * 4 + i],
                                                     psv[i][:], COPYF)

            # ---------------- Phase 2: attention -----------------
            with (
                tc.tile_pool(name="attnT", bufs=1) as attnT_pool,
                tc.tile_pool(name="wo", bufs=1) as wo_pool,
            ):
                attnT = attnT_pool.tile([DK, H_loc, S_], BF16)
                wo_sb = wo_pool.tile([P, H_loc, D], BF16)
                nc.sync.dma_start(wo_sb[:], woT_t[:])

                with (
                    tc.tile_pool(name="expt", bufs=6) as expt,
                    tc.tile_pool(name="accp", bufs=2) as accp,
                    tc.tile_pool(name="sc_ps", bufs=2, space="PSUM") as sc_ps,
                    tc.tile_pool(name="pv_ps", bufs=2, space="PSUM") as pv_ps,
                    tc.tile_pool(name="op_ps", bufs=2, space="PSUM") as op_ps,
                    tc.tile_pool(name="inv", bufs=2) as invp,
                    tc.tile_pool(name="osb", bufs=3) as osb,
                ):
                    M_ext = maskE_sb[:, 0:2 * P]   # zeros | triangle
                    M_tri = maskE_sb[:, P:2 * P]   # triangle only

                    # o_proj for one (s-block, 512-col) output tile; these
                    # units interleave into the NEXT q-block's attention so
                    # the PE never starves while ACT works through the exps
                    NT = min(512, D)
                    def oproj_unit(sb_i, nt):
                        ps = op_ps.tile([P, NT], F32, tag="op",
                                        name=f"op_{sb_i}_{nt}")
                        for ec in range(H_loc):
                            nc.tensor.matmul(
                                ps[:],
                                attnT[:, ec, sb_i * P:(sb_i + 1) * P],
                                wo_sb[:, ec, nt * NT:(nt + 1) * NT],
                                start=(ec == 0), stop=(ec == H_loc - 1))
                        o_nt = osb.tile([P, NT], F32, tag="osb",
                                        name=f"osb_{sb_i}_{nt}")
                        # alternate eviction engines: keeps ACT (exp-bound
                        # on HW) relieved without serializing behind DVE
                        if (sb_i + nt) % 2:
                            nc.vector.tensor_scalar_mul(o_nt[:], ps[:], 1.0)
                        else:
                            nc.scalar.activation(o_nt[:], ps[:], COPYF)
                        nc.sync.dma_start(
                            out[sb_i * P:(sb_i + 1) * P, nt * NT:(nt + 1) * NT],
                            o_nt[:])

                    pending = []
                    drain_k = max(1, -(-((QB // P) * (D // NT)) // H_loc))

                    # qb 0 last: its heads have the least attention work per
                    # head, so they benefit most from o_proj padding
                    qb_order = list(range(1, NQB)) + [0]
                    for qb in qb_order:
                        kcn = (qb + 1) * NDIAG
                        npair = kcn // 2
                        q_lo = qb * QB
                        # per-head accumulation state, created lazily
                        hstate = {}

                        def ensure(h):
                            if h not in hstate:
                                hstate[h] = (
                                    pv_ps.tile([P, QB], F32, tag="pv",
                                               name=f"pv_{qb}_{h}"),
                                    accp.tile([P, QB], BF16, tag="acc",
                                              name=f"acc_{qb}_{h}"))
                            return hstate[h]

                        # pair p covers chunks (2p, 2p+1).
                        #  p <  2*qb : both full
                        #  p == 2*qb : A = (j0 full, j1 full-width+mask)
                        #  p == 2*qb+1: B = (j2, j3) suffix [256:512)
                        def scores_exp(h, p):
                            is_b = (p == 2 * qb + 1)
                            ps_s = sc_ps.tile([P, 2, QB], F32, tag="sc",
                                              name=f"ss_{h}_{qb}_{p}")
                            e_t = expt.tile([P, 2, QB], BF16, tag="e",
                                            name=f"e_{h}_{qb}_{p}")
                            off = 2 * P if is_b else 0
                            for i in range(2):
                                kc = 2 * p + i
                                nc.tensor.matmul(
                                    ps_s[:, i, off:],
                                    kt_res[:, h, kc * P:(kc + 1) * P],
                                    qt_res[:, h, q_lo + off:q_lo + QB],
                                    start=True, stop=True)
                            nc.scalar.activation(e_t[:, :, off:],
                                                 ps_s[:, :, off:], EXPF,
                                                 scale=SCALE)
                            if p == 2 * qb:
                                # pair A: j=0 half gets the triangle on the
                                # block's first 128 cols; j=1 half gets
                                # zeros+triangle on the first 256
                                nc.vector.tensor_tensor(
                                    e_t[:, 0, 0:P], e_t[:, 0, 0:P],
                                    M_tri, MUL)
                                nc.vector.tensor_tensor(
                                    e_t[:, 1, 0:2 * P], e_t[:, 1, 0:2 * P],
                                    M_ext, MUL)
                            elif is_b:
                                nc.vector.tensor_tensor(
                                    e_t[:, 0, 2 * P:3 * P],
                                    e_t[:, 0, 2 * P:3 * P], M_tri, MUL)
                                nc.vector.tensor_tensor(
                                    e_t[:, 1, 2 * P:], e_t[:, 1, 2 * P:],
                                    M_ext, MUL)
                            return e_t

                        def consume(h, p, e_t):
                            # denominator accumulation (DVE, bf16) + PV
                            ps_pv, acc = ensure(h)
                            is_b = (p == 2 * qb + 1)
                            if p == 0:
                                nc.vector.tensor_tensor(
                                    acc[:], e_t[:, 0], e_t[:, 1], ADD)
                            elif is_b:
                                for i in range(2):
                                    nc.vector.tensor_tensor(
                                        acc[:, 2 * P:], acc[:, 2 * P:],
                                        e_t[:, i, 2 * P:], ADD)
                            else:
                                for i in range(2):
                                    nc.vector.tensor_tensor(
                                        acc[:], acc[:], e_t[:, i], ADD)
                            for i in range(2):
                                kc = 2 * p + i
                                v_sl = v_res[:, kc, h * DK:(h + 1) * DK]
                                if is_b:
                                    nc.tensor.matmul(
                                        ps_pv[:, 2 * P:], v_sl,
                                        e_t[:, i, 2 * P:],
                                        start=False, stop=(kc == kcn - 1))
                                elif p == 2 * qb and i == 1:
                                    # pair A, j=1: first 128 cols are
                                    # mask-zeroed, PV can skip them
                                    nc.tensor.matmul(
                                        ps_pv[:, P:], v_sl, e_t[:, 1, P:],
                                        start=False, stop=(kc == kcn - 1))
                                else:
                                    nc.tensor.matmul(
                                        ps_pv[:], v_sl, e_t[:, i],
                                        start=(kc == 0), stop=(kc == kcn - 1))
                            if p == npair - 1:
                                finalize(h)

                        def finalize(h):
                            ps_pv, acc = hstate.pop(h)
                            den_b = invp.tile([P, QB], F32, tag="den",
                                              name=f"den_{h}_{qb}")
                            nc.gpsimd.partition_all_reduce(
                                den_b[:], acc[:], channels=P,
                                reduce_op=bass_isa.ReduceOp.add)
                            inv_b = invp.tile([P, QB], F32, tag="invb",
                                              name=f"inv_{h}_{qb}")
                            nc.vector.reciprocal(inv_b[:], den_b[:])
                            nc.vector.tensor_tensor(
                                attnT[:, h, q_lo:q_lo + QB], ps_pv[:],
                                inv_b[:], MUL)

                        # ONE software pipeline across all heads of this
                        # q-block (two pairs deep), so per-head fill/drain
                        # bubbles vanish; o_proj units of the previous
                        # q-block drain at head starts as ready PE work
                        pipe = []
                        for h in range(H_loc):
                            for p in range(npair):
                                pipe.append((h, p, scores_exp(h, p)))
                                if p == min(1, npair - 1):
                                    # drain behind the freshly issued scores
                                    # so ACT has exp work while PE does these
                                    for _ in range(drain_k):
                                        if pending:
                                            oproj_unit(*pending.pop(0))
                                if len(pipe) > 2:
                                    hh, pp, e_t = pipe.pop(0)
                                    consume(hh, pp, e_t)
                        for hh, pp, e_t in pipe:
                            consume(hh, pp, e_t)

                        # queue this q-block's o_proj output tiles
                        pending.extend(
                            (sb_i, nt)
                            for sb_i in range(qb * (QB // P),
                                              (qb + 1) * (QB // P))
                            for nt in range(D // NT))
                    for u in pending:
                        oproj_unit(*u)

    nc.compile()
    return nc


def make_tables(token_positions, S_=S, DK=D_K):
    """Host-side RoPE tables (de-interleaved halves) + extended causal mask.

    cosH = [cos; cos]; sinN = [+sin; -sin] (see rope_evict).
    """
    pos = np.asarray(token_positions).astype(np.float64)
    half = np.arange(0, DK, 2, dtype=np.float64) / DK
    inv_freq = 1.0 / (ROPE_THETA ** half)  # [DK/2]
    ang = pos[:, None] * inv_freq[None, :]  # [S, DK/2]
    c = np.cos(ang).T.astype(NP_BF16)  # [DK/2, S]
    s = np.sin(ang).T.astype(NP_BF16)
    cosH = np.ascontiguousarray(np.concatenate([c, c], axis=0))  # [DK, S]
    sinN = np.ascontiguousarray(np.concatenate([s, -s], axis=0))
    kl = np.arange(128)[:, None]
    ql = np.arange(128)[None, :]
    tri = (ql >= kl).astype(NP_BF16)  # [128, 128] causal triangle
    maskE = np.concatenate([np.zeros((128, 128), NP_BF16), tri], axis=1)
    return cosH, sinN, np.ascontiguousarray(maskE)


# de-interleave permutation within each head's 128 dims: even dims first
_DEINT = np.concatenate([np.arange(0, D_K, 2), np.arange(1, D_K, 2)])


def deinterleave_cols(wT, n_heads):
    """Permute per-head output columns of a [D, n_heads*DK] matrix so even
    RoPE dims land in rows 0..63 of the head-transposed projection."""
    w = np.asarray(wT)
    out = np.empty_like(w)
    for h in range(n_heads):
        out[:, h * D_K:(h + 1) * D_K] = w[:, h * D_K + _DEINT]
    return out


def head_major(wT, n_heads, d=D_MODEL):
    """[D, E] -> [H, P, KCN*DK]: per-head, partition-major contiguous."""
    kcn = d // 128
    return np.ascontiguousarray(
        wT.reshape(kcn, 128, n_heads, D_K).transpose(2, 1, 0, 3)
        .reshape(n_heads, 128, kcn * D_K))


def make_in_maps(x, token_positions, q_w, k_w, v_w, o_w):
    cosH, sinN, maskE = make_tables(token_positions)
    x = np.asarray(x, np.float32)
    in_maps = []
    for c in range(N_CORES):
        b, g = c // GROUPS, c % GROUPS
        e_lo, e_hi = g * H_LOC * D_K, (g + 1) * H_LOC * D_K
        wqT = deinterleave_cols(np.asarray(q_w, np.float32)[e_lo:e_hi, :].T, H_LOC)
        wkT = deinterleave_cols(np.asarray(k_w, np.float32)[e_lo:e_hi, :].T, H_LOC)
        wqHm = head_major(wqT, H_LOC).astype(NP_BF16)
        wkHm = head_major(wkT, H_LOC).astype(NP_BF16)
        in_maps.append({
            "xT": np.ascontiguousarray(x[b].T).astype(NP_BF16),
            "wqH": wqHm,
            "wkH": wkHm,
            "wvT": np.ascontiguousarray(np.asarray(v_w, np.float32)[e_lo:e_hi, :].T).astype(NP_BF16),
            "woT": np.ascontiguousarray(np.asarray(o_w, np.float32)[:, e_lo:e_hi].T).astype(NP_BF16),
            "cosH": cosH,
            "sinN": sinN,
            "maskE": maskE,
        })
    return in_maps


_NC_CACHE = None


def get_nc():
    global _NC_CACHE
    if _NC_CACHE is None:
        _NC_CACHE = build_nc(D_MODEL, S, H_LOC)
    return _NC_CACHE


def kernel(x, token_positions, q_w, k_w, v_w, o_w):
    from concourse.bass_utils import run_bass_kernel_spmd

    nc = get_nc()
    in_maps = make_in_maps(x, token_positions, q_w, k_w, v_w, o_w)
    res = run_bass_kernel_spmd(nc, in_maps, list(range(N_CORES)))
    outs = [res.results[c]["out"] for c in range(N_CORES)]
    full = np.empty((B, S, D_MODEL), np.float32)
    for b in range(B):
        full[b] = outs[GROUPS * b]
        for g in range(1, GROUPS):
            full[b] += outs[GROUPS * b + g]
    return full
